# revision 9
# baseline (speedup 1.0000x reference)
"""Trainium2 Bass kernel for AFCNet (per-sample 1x1-conv MLP), 8-core data parallel.

Network per sample b (dims 1024 -> 512 -> 256 -> 128 -> 64 -> 1, HW=64):
  q = sigmoid(W1 x + b1); q = q * (drop1 >= .5) * 2
  q = sigmoid(W2 q + b2); q = q * (drop2 >= .5) * 2
  q = sigmoid(W3 q + b3); q = sigmoid(W4 q + b4); out = W5 q + b5

Sharding: batch 64 -> 8 cores x 8 samples (pure data parallel).

Per-core kernel design (vs the fp32/bf16 baseline):
  - W1/W2/W3, x, and the dropout masks ship as fp8e4 (e4m3), W4/W5 and
    activations as bf16: quarters HBM traffic AND doubles LDWEIGHTS
    throughput (FWL). fp8 weights are pre-scaled by power-of-2 factors
    (picked at runtime, range-capped at 224) and compensated via the
    ScalarE activation `scale` operand read from SBUF, so the module
    compiles once. Masks are {0.0, 2.0} (dropout compare + 1/(1-p) done
    on host, exact in fp8).
  - Biases are folded into the matmul accumulation groups: a one-hot
    moving operand against per-sample bias rows as stationary adds each
    sample's bias into PSUM in one MM per (layer, cout-chunk). Bias MMs
    come AFTER the weight MMs of their bank (per-element has_written
    bits turn them into accumulates), so PE can start on w1x0 alone.
  - L1/L2 processed as two half-waves of 4 samples (pipelines with the
    DMA stream); L4/L5 fused across all 8 samples to shorten the serial
    ScalarE<->PE tail. One ACT per (layer, wave) amortizes the
    ~350-cycle ACT overhead.
  - All big blobs ride the scalar-engine HWDGE ring in consumption
    order (it comes out of preamble ~5us before the sync ring; a single
    ring avoids round-robin bandwidth stealing from the earliest blob).
  - A 64-MM warmup accumulation group against a memset tile keeps the
    PE HAM clock-gate busy during the initial DMA wait so real matmuls
    run at 2.4 GHz from the start.
  - PSUM: exactly 8 banks; one start=True per bank per wave.
"""

import time

import ml_dtypes
import numpy as np

import concourse.tile as tile
from concourse import bacc, mybir
from concourse.bass_utils import run_bass_kernel_spmd

N_CORES = 8
S = 8            # samples per core
HW = 64
F8NP = ml_dtypes.float8_e4m3
BFNP = ml_dtypes.bfloat16

BF16 = mybir.dt.bfloat16
F8 = mybir.dt.float8e4
F32 = mybir.dt.float32
SIG = mybir.ActivationFunctionType.Sigmoid
IDENT = mybir.ActivationFunctionType.Identity
MULT = mybir.AluOpType.mult

# --- w1x blob columns (fp8): w1T chunks (k0..7, m0..3)*128, then x chunks ---
X_OFF = 4096                    # 8 chunks x 64
W1X_COLS = 4608
# --- w23m blob columns (fp8) ---
W3_OFF = 1024                   # (k0..1)*128 after w2T (k0..3, m0..1)*128
M1_OFF = 1280                   # mask1 (m0..3, hw)
M2_OFF = 1536                   # mask2 (m0..1, hw)
W23M_COLS = 1664
# --- wb blob (bf16) [128, S*65]: per sample w4T(64) + w5col(1) ---
WB_PER = 65
WB_COLS = S * WB_PER
# --- cb blob (bf16) [8, 3649]: one-hots + scaled biases ---
OH4_OFF = 0                     # [4, 256] one-hot (1.0 at p==j)
OH8_OFF = 256                   # [8, 512] one-hot
B1_OFF = OH8_OFF + 512          # 2h x 2jp blocks of [2, 512]
B2_OFF = B1_OFF + 4 * 512       # 2h x [4, 256]
B3_OFF = B2_OFF + 2 * 256       # 2h x [4, 128]
B4_OFF = B3_OFF + 2 * 128       # [8, 64]
B5_OFF = B4_OFF + 64            # [8, 1]
CB_COLS = B5_OFF + 1

_COMPILED = None
LAST_RESULT = None


def _build():
    nc = bacc.Bacc(target_bir_lowering=False)
    w1x_d = nc.declare_dram_parameter("w1x", [S, 128, W1X_COLS], F8, isOutput=False)
    w23m_d = nc.declare_dram_parameter("w23m", [S, 128, W23M_COLS], F8, isOutput=False)
    wb_d = nc.declare_dram_parameter("wb", [128, WB_COLS], BF16, isOutput=False)
    cb_d = nc.declare_dram_parameter("cb", [8, CB_COLS], BF16, isOutput=False)
    sc_d = nc.declare_dram_parameter("sc", [128, 3], F32, isOutput=False)
    out_d = nc.declare_dram_parameter("out", [1, S * HW], F32, isOutput=True)

    with tile.TileContext(nc) as tc:
        with (
            tc.tile_pool(name="sbuf", bufs=1) as sb,
            tc.tile_pool(name="psum", bufs=1, space="PSUM") as ps,
        ):
            cbT = sb.tile([8, CB_COLS], BF16, tag="cbT")
            wbT = sb.tile([128, WB_COLS], BF16, tag="wbT")
            scT = sb.tile([128, 3], F32, tag="scT")
            # small blobs on the sync HWDGE ring (they'd serialize ~2us
            # each in front of the big blobs if put on the same ring)
            nc.sync.dma_start(out=cbT[:], in_=cb_d[:, :])
            nc.sync.dma_start(out=scT[:], in_=sc_d[:, :])
            nc.sync.dma_start(out=wbT[:], in_=wb_d[:, :])

            w1xT = []
            w23mT = []
            for j in range(S):
                w1xT.append(sb.tile([128, W1X_COLS], F8, tag=f"w1x{j}",
                                    name=f"w1x{j}"))
                w23mT.append(sb.tile([128, W23M_COLS], F8, tag=f"w23m{j}",
                                     name=f"w23m{j}"))
            # All big blobs on ONE HWDGE ring (scalar engine — out of
            # preamble earliest) in exact consumption order.
            for h in range(2):
                for j in range(4):
                    nc.scalar.dma_start(out=w1xT[4 * h + j][:],
                                        in_=w1x_d[4 * h + j, :, :])
                for j in range(4):
                    nc.scalar.dma_start(out=w23mT[4 * h + j][:],
                                        in_=w23m_d[4 * h + j, :, :])

            oh4 = cbT[0:4, OH4_OFF:OH4_OFF + 256]
            oh8 = cbT[0:8, OH8_OFF:OH8_OFF + 512]

            # activations (bf16)
            q1 = [sb.tile([128, 4, 4, HW], BF16, tag=f"q1_{h}",
                          name=f"q1_{h}") for h in range(2)]
            q2 = [sb.tile([128, 4, 2, HW], BF16, tag=f"q2_{h}",
                          name=f"q2_{h}") for h in range(2)]
            q3 = [sb.tile([128, 4, HW], BF16, tag=f"q3_{h}",
                          name=f"q3_{h}") for h in range(2)]
            q4 = sb.tile([64, S, HW], BF16, tag="q4")
            out_sb = sb.tile([1, S, HW], F32, tag="out_sb")

            # PSUM: exactly 8 banks
            p1 = [ps.tile([128, 4, 4, HW], F32, tag=f"p1_{h}",
                          name=f"p1_{h}") for h in range(2)]
            p2 = ps.tile([128, 4, 2, HW], F32, tag="p2")
            p3 = ps.tile([128, 4, HW], F32, tag="p3",
                         padded_shape=[128, 4, 2 * HW])
            p4 = ps.tile([64, S, HW], F32, tag="p4")
            p5 = ps.tile([1, S, HW], F32, tag="p5")

            # PE warmup: one long accumulation group (a single bank clear;
            # back-to-back start=True groups into one bank can wedge the
            # clear-vs-drain path) keeps HAM from throttling the PE while
            # the first weight DMA is in flight.
            warm = sb.tile([128, HW], BF16, tag="warm")
            nc.vector.memset(warm[:], 0.0)
            for i in range(64):
                nc.tensor.matmul(p5[:, 0, :], warm[:, 0:1], warm[:],
                                 start=(i == 0), stop=(i == 63),
                                 skip_group_check=True)

            sc1 = scT[:, 0:1]
            sc2 = scT[:, 1:2]
            sc3 = scT[:, 2:3]

            # ---- layer 1: 1024 -> 512 (4 cout chunks), per half-wave ----
            for h in range(2):
                ph = p1[h]
                for jp in range(2):  # one PSUM bank per 2 samples
                    for jl in range(2):
                        j = 2 * jp + jl
                        wt = w1xT[4 * h + j]
                        for m in range(4):
                            for k in range(8):
                                nc.tensor.matmul(
                                    ph[:, j, m, :],
                                    wt[:, (k * 4 + m) * 128:(k * 4 + m + 1) * 128],
                                    wt[:, X_OFF + k * HW:X_OFF + (k + 1) * HW],
                                    start=(jl == 0 and m == 0 and k == 0),
                                    stop=False,
                                    skip_group_check=True,
                                )
                    b1blk = cbT[0:2, B1_OFF + (2 * h + jp) * 512:
                                     B1_OFF + (2 * h + jp + 1) * 512]
                    for m in range(4):
                        nc.tensor.matmul(
                            ph[:, 2 * jp:2 * jp + 2, m, :],
                            b1blk[:, m * 128:(m + 1) * 128],
                            oh4[0:2, 0:128],
                            start=False, stop=(m == 3), skip_group_check=True,
                        )

            for h in range(2):
                nc.scalar.activation(q1[h][:, :, :, :], p1[h][:, :, :, :],
                                     SIG, scale=sc1)
                for j in range(4):
                    mk = w23mT[4 * h + j][:, M1_OFF:M1_OFF + 256]
                    nc.vector.scalar_tensor_tensor(
                        out=q1[h][:, j, :, :],
                        in0=mk.rearrange("p (m t) -> p m t", m=4),
                        scalar=1.0,
                        in1=q1[h][:, j, :, :],
                        op0=MULT, op1=MULT,
                    )

            # ---- layer 2: 512 -> 256 (2 cout chunks), per half-wave ----
            for h in range(2):
                for j in range(4):
                    wt = w23mT[4 * h + j]
                    for m in range(2):
                        for k in range(4):
                            nc.tensor.matmul(
                                p2[:, j, m, :],
                                wt[:, (k * 2 + m) * 128:(k * 2 + m + 1) * 128],
                                q1[h][:, j, k, :],
                                start=(j == 0 and m == 0 and k == 0),
                                stop=False,
                                skip_group_check=True,
                            )
                b2blk = cbT[0:4, B2_OFF + h * 256:B2_OFF + (h + 1) * 256]
                for m in range(2):
                    nc.tensor.matmul(
                        p2[:, :, m, :], b2blk[:, m * 128:(m + 1) * 128], oh4[:],
                        start=False, stop=(m == 1), skip_group_check=True,
                    )
                nc.scalar.activation(q2[h][:, :, :, :], p2[:, :, :, :],
                                     SIG, scale=sc2)
                for j in range(4):
                    mk = w23mT[4 * h + j][:, M2_OFF:M2_OFF + 128]
                    nc.vector.scalar_tensor_tensor(
                        out=q2[h][:, j, :, :],
                        in0=mk.rearrange("p (m t) -> p m t", m=2),
                        scalar=1.0,
                        in1=q2[h][:, j, :, :],
                        op0=MULT, op1=MULT,
                    )

            # ---- layer 3: 256 -> 128, per half-wave ----
            for h in range(2):
                for j in range(4):
                    wt = w23mT[4 * h + j]
                    for k in range(2):
                        nc.tensor.matmul(
                            p3[:, j, :],
                            wt[:, W3_OFF + k * 128:W3_OFF + (k + 1) * 128],
                            q2[h][:, j, k, :],
                            start=(j == 0 and k == 0), stop=False,
                            skip_group_check=True,
                        )
                b3blk = cbT[0:4, B3_OFF + h * 128:B3_OFF + (h + 1) * 128]
                nc.tensor.matmul(p3[:, :, :], b3blk[:], oh4[:],
                                 start=False, stop=True, skip_group_check=True)
                nc.scalar.activation(q3[h][:, :, :], p3[:, :, :],
                                     SIG, scale=sc3)

            # ---- layer 4: 128 -> 64, fused across all 8 samples ----
            for j in range(S):
                nc.tensor.matmul(
                    p4[:, j, :],
                    wbT[:, j * WB_PER:j * WB_PER + 64],
                    q3[j // 4][:, j % 4, :],
                    start=(j == 0), stop=False, skip_group_check=True,
                )
            nc.tensor.matmul(p4[:, :, :], cbT[0:8, B4_OFF:B4_OFF + 64], oh8[:],
                             start=False, stop=True, skip_group_check=True)
            nc.scalar.activation(q4[:, :, :], p4[:, :, :], SIG)

            # ---- layer 5: 64 -> 1, fused across all 8 samples ----
            for j in range(S):
                nc.tensor.matmul(
                    p5[:, j, :],
                    wbT[0:64, j * WB_PER + 64:j * WB_PER + 65],
                    q4[:, j, :],
                    start=(j == 0), stop=False, skip_group_check=True,
                )
            nc.tensor.matmul(p5[:, :, :], cbT[0:8, B5_OFF:B5_OFF + 1], oh8[:],
                             start=False, stop=True, skip_group_check=True)
            nc.scalar.activation(out_sb[:, :, :], p5[:, :, :], IDENT)

            nc.sync.dma_start(out=out_d[:, :],
                              in_=out_sb.rearrange("p a b -> p (a b)"))
    nc.compile()
    return nc


def _pow2_scale(a, cap=224.0):
    m = float(np.abs(a).max())
    if m == 0.0:
        return 1.0
    return float(2.0 ** np.floor(np.log2(cap / m)))


def _pack(x, w1, b1, w2, b2, w3, b3, w4, b4, w5, b5, drop1, drop2):
    """Build per-sample w1x/w23m/wb blobs; return bias/scale data."""
    B = x.shape[0]
    f4 = np.float32
    x3 = np.ascontiguousarray(x.reshape(B, 1024, HW), dtype=f4)
    w1m = w1.reshape(B, 512, 1024).astype(f4, copy=False)
    w2m = w2.reshape(B, 256, 512).astype(f4, copy=False)
    w3m = w3.reshape(B, 128, 256).astype(f4, copy=False)
    w4m = w4.reshape(B, 64, 128).astype(f4, copy=False)
    w5m = w5.reshape(B, 64).astype(f4, copy=False)

    sx = _pow2_scale(x3)
    s1 = _pow2_scale(w1m)
    s2 = _pow2_scale(w2m)
    s3 = _pow2_scale(w3m)

    def chunkT(wT, nk, nm):  # [B, cin, cout] -> [B, 128, nk*nm*128]
        Bn, cin, cout = wT.shape
        return np.ascontiguousarray(
            wT.reshape(Bn, nk, 128, nm, 128).transpose(0, 2, 1, 3, 4)
        ).reshape(Bn, 128, nk * nm * 128)

    w1T = chunkT(np.swapaxes(w1m, 1, 2) * s1, 8, 4)
    xc = np.ascontiguousarray(
        x3.reshape(B, 8, 128, HW).transpose(0, 2, 1, 3)).reshape(B, 128, 512) * sx
    w1x = np.concatenate([w1T, xc], axis=2).astype(F8NP)

    w2T = chunkT(np.swapaxes(w2m, 1, 2) * s2, 4, 2)
    w3T = chunkT(np.swapaxes(w3m, 1, 2) * s3, 2, 1)
    m1 = (drop1.reshape(B, 512, HW) >= np.float32(0.5)).astype(f4) * f4(2.0)
    m1 = np.ascontiguousarray(
        m1.reshape(B, 4, 128, HW).transpose(0, 2, 1, 3)).reshape(B, 128, 256)
    m2 = (drop2.reshape(B, 256, HW) >= np.float32(0.5)).astype(f4) * f4(2.0)
    m2 = np.ascontiguousarray(
        m2.reshape(B, 2, 128, HW).transpose(0, 2, 1, 3)).reshape(B, 128, 128)
    w23m = np.concatenate([w2T, w3T, m1, m2], axis=2).astype(F8NP)

    wb = np.zeros((B, 128, WB_PER), f4)
    wb[:, :, 0:64] = np.swapaxes(w4m, 1, 2)
    wb[:, :64, 64] = w5m

    b1s = b1.astype(f4) * f4(s1 * sx)
    b2s = b2.astype(f4) * f4(s2)
    b3s = b3.astype(f4) * f4(s3)
    scales = (1.0 / (s1 * sx), 1.0 / s2, 1.0 / s3)
    return w1x, w23m, wb, (b1s, b2s, b3s, b4.astype(f4), b5.reshape(B).astype(f4)), scales


def kernel(**inputs):
    global _COMPILED, LAST_RESULT
    if _COMPILED is None:
        _COMPILED = _build()
    nc = _COMPILED

    w1x, w23m, wb, (b1s, b2s, b3s, b4f, b5f), scales = _pack(
        **{k: np.asarray(v) for k, v in inputs.items()})

    oh4 = np.zeros((4, 256), np.float32)
    for p in range(4):
        oh4[p, p * HW:(p + 1) * HW] = 1.0
    oh8 = np.zeros((8, 512), np.float32)
    for p in range(8):
        oh8[p, p * HW:(p + 1) * HW] = 1.0

    in_maps = []
    for c in range(N_CORES):
        sl = slice(c * S, (c + 1) * S)
        wbc = wb[sl].transpose(1, 0, 2).reshape(128, S * WB_PER)

        cb = np.zeros((8, CB_COLS), np.float32)
        cb[0:4, OH4_OFF:OH4_OFF + 256] = oh4
        cb[:, OH8_OFF:OH8_OFF + 512] = oh8
        for h in range(2):
            base = c * S + 4 * h
            for jp in range(2):
                cb[0:2, B1_OFF + (2 * h + jp) * 512:
                        B1_OFF + (2 * h + jp + 1) * 512] = \
                    b1s[base + 2 * jp:base + 2 * jp + 2]
            cb[0:4, B2_OFF + h * 256:B2_OFF + (h + 1) * 256] = b2s[base:base + 4]
            cb[0:4, B3_OFF + h * 128:B3_OFF + (h + 1) * 128] = b3s[base:base + 4]
        cb[:, B4_OFF:B4_OFF + 64] = b4f[c * S:(c + 1) * S]
        cb[:, B5_OFF] = b5f[c * S:(c + 1) * S]

        scc = np.empty((128, 3), np.float32)
        scc[:, 0] = scales[0]
        scc[:, 1] = scales[1]
        scc[:, 2] = scales[2]

        in_maps.append({
            "w1x": np.ascontiguousarray(w1x[sl]),
            "w23m": np.ascontiguousarray(w23m[sl]),
            "wb": np.ascontiguousarray(wbc).astype(BFNP),
            "cb": cb.astype(BFNP),
            "sc": scc,
        })

    res = None
    for attempt in range(3):
        try:
            res = run_bass_kernel_spmd(nc, in_maps, core_ids=list(range(N_CORES)))
            break
        except Exception:
            if attempt == 2:
                raise
            time.sleep(20)
            try:  # best-effort device reconnect after NRT_EXEC_UNIT_UNRECOVERABLE
                import jax
                jax.clear_caches()
                import jax.extend.backend as _jeb
                _jeb.clear_backends()
            except Exception:
                pass
    LAST_RESULT = res
    outs = [np.asarray(res.results[c]["out"]).reshape(S, 8, 8)
            for c in range(N_CORES)]
    return np.concatenate(outs, axis=0).astype(np.float32)
